# revision 7
# baseline (speedup 1.0000x reference)
"""Trainium2 Bass kernel for a 2-layer DeepAR-style LSTM (T=96, B=256, H=512).

Strategy: 8-way data parallel over batch (32 rows/core), zero in-kernel
collectives.  Each core runs the full 96-step recurrence with all weights
resident in SBUF.  Per-step loss terms are emitted as per-step partial
sums/counts; the cross-core combine (the "all-reduce at the end") happens on
the host after gather.

Layout choices:
  - gates are computed as [batch=32 partitions, 2048 free] in PSUM, with the
    gate columns permuted to [i, f, o, g] so sigmoid runs as one [32,1536] op
    and tanh as one [32,512] op per layer.
  - the recurrent matmuls need h transposed ([hidden, batch] as lhsT); h is
    transposed each step on the PE (4x 32x128 transposes into one PSUM bank).
  - layer-0 input x is pre-transposed on the host into [86, T*32] SBUF rows:
    row 0 = raw z, rows 1..84 = covariates+embedding, row 85 = ones (bias0).
    Teacher forcing patches row 0 in place (copy_predicated with mu_{t-1})
    before the K=86 x-matmul of each step.
  - the Gaussian head is deinterleaved on the host: mu = h0.a + h1.b + bias,
    computed as 8 accumulating K=128, N=32 matmuls against h0T/h1T.
  - softplus/ln (different ACT table sets) are deferred to one end-of-kernel
    batch over all 96 steps so the recurrence only ever uses sigmoid/tanh
    (same table set -> zero table reloads in the hot loop).
"""

import os
import sys

import numpy as np

for _p in ("/opt/trn_rl_repo", "/root/.axon_site/_ro/trn_rl_repo"):
    if os.path.isdir(_p) and _p not in sys.path:
        sys.path.insert(0, _p)

import concourse.bass as bass
import concourse.mybir as mybir
from concourse import bacc
from concourse.masks import make_identity
from concourse.tile import TileContext

AF = mybir.ActivationFunctionType
OP = mybir.AluOpType
F32 = mybir.dt.float32
F32R = mybir.dt.float32r

T, B, H, NCORES = 96, 256, 512, 8
BC = B // NCORES          # 32 batch rows per core
XR = 86                   # z + 20 covariates + 64 embedding + ones
G = 4 * H                 # 2048 gate columns
KT = H // 128             # 4 k-tiles of the hidden dim
# gate columns permuted to [i, f, o, g]
PERM = np.concatenate([np.r_[0:512], np.r_[512:1024], np.r_[1536:2048], np.r_[1024:1536]])
LOG2PI = float(np.log(2.0 * np.pi))


def build_nc(t_steps=T, mm_f32r=False):
    nc = bacc.Bacc(None, target_bir_lowering=False)
    TS = t_steps
    N32 = TS * BC

    xt_d = nc.declare_dram_parameter("xt", [XR, N32], F32, isOutput=False)
    labs_d = nc.declare_dram_parameter("labs", [1, N32], F32, isOutput=False)
    whh0_d = nc.declare_dram_parameter("whh0", [128, KT, G], F32, isOutput=False)
    whh1_d = nc.declare_dram_parameter("whh1", [128, KT, G], F32, isOutput=False)
    wih1_d = nc.declare_dram_parameter("wih1", [128, KT, G], F32, isOutput=False)
    wxz_d = nc.declare_dram_parameter("wxz", [XR, G], F32, isOutput=False)
    b1_d = nc.declare_dram_parameter("b1", [1, G], F32, isOutput=False)
    hwmu_d = nc.declare_dram_parameter("hwmu", [128, 2 * KT], F32, isOutput=False)
    hwps_d = nc.declare_dram_parameter("hwps", [128, 2 * KT], F32, isOutput=False)
    hbmu_d = nc.declare_dram_parameter("hbmu", [1, 1], F32, isOutput=False)
    hbps_d = nc.declare_dram_parameter("hbps", [1, 1], F32, isOutput=False)
    hc_d = nc.declare_dram_parameter("hc", [128, H], F32, isOutput=False)

    ohc_d = nc.declare_dram_parameter("out_hc", [128, H], F32, isOutput=True)
    oms_d = nc.declare_dram_parameter("out_musig", [2, BC], F32, isOutput=True)
    olp_d = nc.declare_dram_parameter("out_loss", [2, TS], F32, isOutput=True)

    # bitcast matmul operands to float32r (1 cyc/row) when requested
    mm = (lambda ap: ap.bitcast(F32R)) if mm_f32r else (lambda ap: ap)

    with TileContext(nc) as tc:
        with (
            tc.tile_pool(name="wp", bufs=1) as wp,
            tc.tile_pool(name="st", bufs=1) as st,
            tc.tile_pool(name="wk", bufs=2) as wk,
            tc.tile_pool(name="fin", bufs=2) as fp,
            tc.tile_pool(name="ps", bufs=2, space="PSUM") as ps,
        ):
            # ---- resident inputs ----
            xt = wp.tile_from(xt_d[:, :], name="xt_sb")
            whh0 = wp.tile_from(whh0_d[:, :, :], name="whh0_sb")
            whh1 = wp.tile_from(whh1_d[:, :, :], name="whh1_sb")
            wih1 = wp.tile_from(wih1_d[:, :, :], name="wih1_sb")
            wxz = wp.tile_from(wxz_d[:, :], name="wxz_sb")
            b1 = wp.tile_from(b1_d[:, :], name="b1_sb")
            hwmu = wp.tile_from(hwmu_d[:, :], name="hwmu_sb")
            hwps = wp.tile_from(hwps_d[:, :], name="hwps_sb")
            hbmu = wp.tile_from(hbmu_d[:, :], name="hbmu_sb")
            hbps = wp.tile_from(hbps_d[:, :], name="hbps_sb")

            ident = wp.tile([BC, BC], F32, name="ident")
            make_identity(nc, ident)
            ones1 = wp.tile([1, BC], F32, name="ones1")
            nc.vector.memset(ones1, 1.0)

            # ---- persistent state ----
            mubuf = st.tile([1, N32], F32, name="mubuf")
            psbuf = st.tile([1, N32], F32, name="psbuf")
            ssum = st.tile([1, TS], F32, name="ssum")
            cnt = st.tile([1, TS], F32, name="cnt")
            c0 = st.tile([BC, H], F32, name="c0")
            c1 = st.tile([BC, H], F32, name="c1")
            nc.sync.dma_start(out=c0, in_=hc_d[64:96, :])
            nc.sync.dma_start(out=c1, in_=hc_d[96:128, :])

            h0 = wk.tile([BC, H], F32, name="h0", tag="h0")
            h1 = wk.tile([BC, H], F32, name="h1", tag="h1")
            nc.sync.dma_start(out=h0, in_=hc_d[0:32, :])
            nc.sync.dma_start(out=h1, in_=hc_d[32:64, :])

            def transpose_h(h, label):
                tx = ps.tile([128, KT * BC], F32, tag="small", name=f"tx{label}")
                for k in range(KT):
                    nc.tensor.transpose(
                        tx[:, k * BC:(k + 1) * BC], h[:, k * 128:(k + 1) * 128], ident
                    )
                hT = wk.tile([128, KT * BC], F32, tag=f"hT{label[0]}", name=f"hT{label}")
                nc.vector.tensor_copy(hT, tx)
                return hT

            h0T = transpose_h(h0, "0i")
            h1T = transpose_h(h1, "1i")

            IFO, GSL = slice(0, 1536), slice(1536, 2048)

            def gate_matmuls(gifo, gg, lhsT, w, start, stop):
                """accumulate lhsT.T @ w[k] into [gifo | gg] for 4 k-tiles"""
                for k in range(KT):
                    lhs = mm(lhsT[:, k * BC:(k + 1) * BC])
                    st_ = start and k == 0
                    sp_ = stop and k == KT - 1
                    for j in range(3):
                        nc.tensor.matmul(
                            gifo[:, j * 512:(j + 1) * 512], lhs,
                            mm(w[:, k, j * 512:(j + 1) * 512]),
                            start=st_, stop=sp_,
                        )
                    nc.tensor.matmul(gg, lhs, mm(w[:, k, GSL]), start=st_, stop=sp_)

            def rank1_matmuls(gifo, gg, lhs_row, w_row, start, stop):
                for j in range(3):
                    nc.tensor.matmul(
                        gifo[:, j * 512:(j + 1) * 512], mm(lhs_row),
                        mm(w_row[:, j * 512:(j + 1) * 512]), start=start, stop=stop,
                    )
                nc.tensor.matmul(gg, mm(lhs_row), mm(w_row[:, GSL]), start=start, stop=stop)

            def cell_update(gifo, gg, sifo, tg, tcell, c, label):
                """sigmoid/tanh + c/h update; returns new h tile"""
                nc.scalar.activation(sifo, gifo, AF.Sigmoid)
                nc.scalar.activation(tg, gg, AF.Tanh)
                nc.vector.tensor_tensor(tg, sifo[:, 0:512], tg, OP.mult)        # i*g
                nc.vector.tensor_tensor(c, sifo[:, 512:1024], c, OP.mult)       # f*c
                nc.vector.tensor_tensor(c, c, tg, OP.add)
                nc.scalar.activation(tcell, c, AF.Tanh)
                h_new = wk.tile([BC, H], F32, tag=f"h{label}", name=f"h{label}_t")
                nc.vector.tensor_tensor(h_new, sifo[:, 1024:1536], tcell, OP.mult)
                return h_new

            for t in range(TS):
                tcols = slice(t * BC, (t + 1) * BC)

                # ---- gates layer 0: Whh0 @ h0T (+ x/z via K=86 matmul) ----
                gifo0 = ps.tile([BC, 1536], F32, tag="gifo", name="gifo0")
                gg0 = ps.tile([BC, 512], F32, tag="small", name="gg0")
                gate_matmuls(gifo0, gg0, h0T, whh0, start=True, stop=False)

                if t > 0:
                    zrow = xt[0:1, tcols]
                    msk = wk.tile([1, BC], mybir.dt.int32, tag="msk", name="msk")
                    nc.vector.tensor_scalar(msk, zrow, 0.0, None, OP.is_equal)
                    nc.vector.copy_predicated(
                        zrow, msk, mubuf[0:1, (t - 1) * BC:t * BC]
                    )
                xl = xt[:, tcols]
                for j in range(3):
                    nc.tensor.matmul(
                        gifo0[:, j * 512:(j + 1) * 512], mm(xl),
                        mm(wxz[:, j * 512:(j + 1) * 512]), start=False, stop=True,
                    )
                nc.tensor.matmul(gg0, mm(xl), mm(wxz[:, GSL]), start=False, stop=True)

                # ---- gates layer 1 (part A): Whh1 @ h1T + bias1 ----
                gifo1 = ps.tile([BC, 1536], F32, tag="gifo", name="gifo1")
                gg1 = ps.tile([BC, 512], F32, tag="small", name="gg1")
                gate_matmuls(gifo1, gg1, h1T, whh1, start=True, stop=False)
                rank1_matmuls(gifo1, gg1, ones1, b1, start=False, stop=False)

                # ---- layer 0 activations & state ----
                sifo0 = wk.tile([BC, 1536], F32, tag="sifo", name="sifo0")
                tg0 = wk.tile([BC, 512], F32, tag="tg", name="tg0")
                tc0 = wk.tile([BC, 512], F32, tag="tc", name="tc0")
                h0 = cell_update(gifo0, gg0, sifo0, tg0, tc0, c0, "0")
                h0T = transpose_h(h0, f"0_{t}")

                # ---- gates layer 1 (part B): Wih1 @ h0T ----
                gate_matmuls(gifo1, gg1, h0T, wih1, start=False, stop=True)

                # ---- layer 1 activations & state ----
                sifo1 = wk.tile([BC, 1536], F32, tag="sifo", name="sifo1")
                tg1 = wk.tile([BC, 512], F32, tag="tg", name="tg1")
                tc1 = wk.tile([BC, 512], F32, tag="tc", name="tc1")
                h1 = cell_update(gifo1, gg1, sifo1, tg1, tc1, c1, "1")
                h1T = transpose_h(h1, f"1_{t}")

                # ---- head: mu/ps rows via 8 accumulating K=128,N=32 matmuls ----
                hdmu = ps.tile([1, BC], F32, tag="small", name="hdmu")
                hdps = ps.tile([1, BC], F32, tag="small", name="hdps")
                for j in range(2 * KT):
                    hT = h0T if j < KT else h1T
                    rhs = hT[:, (j % KT) * BC:(j % KT + 1) * BC]
                    nc.tensor.matmul(
                        hdmu, mm(hwmu[:, j:j + 1]), mm(rhs),
                        start=(j == 0), stop=(j == 2 * KT - 1),
                    )
                    nc.tensor.matmul(
                        hdps, mm(hwps[:, j:j + 1]), mm(rhs),
                        start=(j == 0), stop=(j == 2 * KT - 1),
                    )
                nc.vector.tensor_scalar(mubuf[0:1, tcols], hdmu, hbmu, None, OP.add)
                nc.vector.tensor_scalar(psbuf[0:1, tcols], hdps, hbps, None, OP.add)

            # ---- deferred loss math (softplus/ln batched; 1 table swap each) ----
            NCH = 8 if TS % 8 == 0 else 1
            SPC = TS // NCH          # steps per chunk
            W = SPC * BC
            sig_last = None
            for ci in range(NCH):
                cc = slice(ci * W, (ci + 1) * W)
                # sigma = softplus(ps) = ln(1 + exp(ps));  Exp and Ln share a table set
                sig = fp.tile([1, W], F32, tag="fsig", name="sig")
                t1 = fp.tile([1, W], F32, tag="ft1", name="t1")
                nc.scalar.activation(t1, psbuf[0:1, cc], AF.Exp)
                nc.vector.tensor_scalar(t1, t1, 1.0, None, OP.add)
                nc.scalar.activation(sig, t1, AF.Ln)
                nc.vector.reciprocal(t1, sig)
                lb = fp.tile([1, W], F32, tag="flb", name="lb")
                nc.sync.dma_start(out=lb, in_=labs_d[0:1, cc])
                t2 = fp.tile([1, W], F32, tag="ft2", name="t2")
                nc.vector.tensor_tensor(t2, lb, mubuf[0:1, cc], OP.subtract)
                nc.vector.tensor_tensor(t2, t2, t1, OP.mult)
                nc.vector.tensor_tensor(t2, t2, t2, OP.mult)          # ((lab-mu)/sig)^2
                t3 = fp.tile([1, W], F32, tag="ft3", name="t3")
                nc.scalar.activation(t3, sig, AF.Ln)
                nc.vector.scalar_tensor_tensor(t2, t2, -0.5, t3, OP.mult, OP.subtract)
                nc.vector.tensor_scalar(t3, lb, 0.0, None, OP.not_equal)  # mask
                nc.vector.scalar_tensor_tensor(t2, t2, -0.5 * LOG2PI, t3, OP.add, OP.mult)
                csl = slice(ci * SPC, (ci + 1) * SPC)
                nc.vector.tensor_reduce(
                    ssum[0:1, csl], t2.rearrange("p (t b) -> p t b", b=BC),
                    mybir.AxisListType.X, OP.add,
                )
                nc.vector.tensor_reduce(
                    cnt[0:1, csl], t3.rearrange("p (t b) -> p t b", b=BC),
                    mybir.AxisListType.X, OP.add,
                )
                sig_last = sig

            # ---- outputs ----
            nc.sync.dma_start(out=ohc_d[0:32, :], in_=h0)
            nc.sync.dma_start(out=ohc_d[32:64, :], in_=h1)
            nc.sync.dma_start(out=ohc_d[64:96, :], in_=c0)
            nc.sync.dma_start(out=ohc_d[96:128, :], in_=c1)
            nc.sync.dma_start(out=oms_d[0:1, :], in_=mubuf[0:1, (TS - 1) * BC:TS * BC])
            nc.sync.dma_start(out=oms_d[1:2, :], in_=sig_last[0:1, (SPC - 1) * BC:SPC * BC])
            nc.sync.dma_start(out=olp_d[0:1, :], in_=ssum)
            nc.sync.dma_start(out=olp_d[1:2, :], in_=cnt)

    nc.compile()
    return nc


# ------------------------- host side -------------------------

def _pack_w(Wt_perm):
    """[512, 2048] (already col-permuted W.T) -> [128, 4, 2048] k-tile major"""
    return np.ascontiguousarray(
        Wt_perm.reshape(KT, 128, G).transpose(1, 0, 2), np.float32
    )


def host_prep(inputs, t_steps=T):
    f = lambda k: np.asarray(inputs[k], np.float32)
    train_batch, labels = f("train_batch"), f("labels_batch")
    hidden, cell, embed = f("hidden"), f("cell"), f("embed")
    W_ih0, W_hh0 = f("W_ih0"), f("W_hh0")
    W_ih1, W_hh1 = f("W_ih1"), f("W_hh1")
    b_ih0, b_hh0 = f("b_ih0"), f("b_hh0")
    b_ih1, b_hh1 = f("b_ih1"), f("b_hh1")
    W_mu, b_mu, W_ps, b_ps = f("W_mu"), f("b_mu"), f("W_ps"), f("b_ps")
    idx = np.asarray(inputs["idx"])
    TS = t_steps

    emb = embed[idx[0]]                                    # [B, 64]
    shared = {
        "whh0": _pack_w(np.ascontiguousarray(W_hh0.T[:, PERM])),
        "whh1": _pack_w(np.ascontiguousarray(W_hh1.T[:, PERM])),
        "wih1": _pack_w(np.ascontiguousarray(W_ih1.T[:, PERM])),
        "wxz": np.ascontiguousarray(
            np.concatenate([W_ih0.T[:, PERM], (b_ih0 + b_hh0)[None, PERM]], 0),
            np.float32),
        "b1": np.ascontiguousarray((b_ih1 + b_hh1)[None, PERM], np.float32),
        "hwmu": np.ascontiguousarray(np.concatenate(
            [W_mu[0, 0::2].reshape(KT, 128).T, W_mu[0, 1::2].reshape(KT, 128).T], 1)),
        "hwps": np.ascontiguousarray(np.concatenate(
            [W_ps[0, 0::2].reshape(KT, 128).T, W_ps[0, 1::2].reshape(KT, 128).T], 1)),
        "hbmu": b_mu.reshape(1, 1),
        "hbps": b_ps.reshape(1, 1),
    }

    in_maps = []
    for core in range(NCORES):
        sl = slice(core * BC, (core + 1) * BC)
        xc = train_batch[:TS, sl, :]                       # [TS, 32, 21]
        xt = np.empty((XR, TS, BC), np.float32)
        xt[0] = xc[:, :, 0]
        xt[1:21] = xc[:, :, 1:].transpose(2, 0, 1)
        xt[21:85] = np.broadcast_to(emb[sl].T[:, None, :], (64, TS, BC))
        xt[85] = 1.0
        m = dict(shared)
        m["xt"] = np.ascontiguousarray(xt.reshape(XR, TS * BC))
        m["labs"] = np.ascontiguousarray(labels[:TS, sl].reshape(1, TS * BC))
        m["hc"] = np.ascontiguousarray(np.concatenate(
            [hidden[0, sl], hidden[1, sl], cell[0, sl], cell[1, sl]], 0))
        in_maps.append(m)
    return in_maps


def assemble(results, t_steps=T):
    mus = np.concatenate([r["out_musig"][0] for r in results]).astype(np.float32)
    sigmas = np.concatenate([r["out_musig"][1] for r in results]).astype(np.float32)
    h = np.stack([
        np.concatenate([r["out_hc"][0:32] for r in results], 0),
        np.concatenate([r["out_hc"][32:64] for r in results], 0),
    ]).astype(np.float32)
    c = np.stack([
        np.concatenate([r["out_hc"][64:96] for r in results], 0),
        np.concatenate([r["out_hc"][96:128] for r in results], 0),
    ]).astype(np.float32)
    S = np.sum([r["out_loss"][0] for r in results], axis=0, dtype=np.float32)
    CNT = np.sum([r["out_loss"][1] for r in results], axis=0, dtype=np.float32)
    n = np.maximum(CNT, 1.0).astype(np.float32)
    loss = np.float32(np.sum(-(S / n), dtype=np.float32))
    return mus, sigmas, h, c, np.array([loss], np.float32)


_CACHE = {}
LAST_EXEC_NS = None


def kernel(**inputs):
    global LAST_EXEC_NS
    from concourse.bass_utils import run_bass_kernel_spmd

    mm_f32r = os.environ.get("LSTM_MM_F32R", "0") == "1"
    trace = os.environ.get("LSTM_TRACE", "0") == "1"
    key = (T, mm_f32r)
    if key not in _CACHE:
        _CACHE[key] = build_nc(T, mm_f32r=mm_f32r)
    nc = _CACHE[key]
    in_maps = host_prep(inputs, T)
    res = run_bass_kernel_spmd(nc, in_maps, core_ids=list(range(NCORES)), trace=trace)
    LAST_EXEC_NS = res.exec_time_ns
    return assemble(res.results, T)


# revision 14
# speedup vs baseline: 2.9573x; 2.9573x over previous
"""Trainium2 Bass kernel for a 2-layer DeepAR-style LSTM (T=96, B=256, H=512).

Strategy: 8-way data parallel over batch (32 rows/core), zero in-kernel
collectives.  Each core runs the full 96-step recurrence with all weights
resident in SBUF.  Per-step loss terms are emitted as per-step partial
sums/counts; the cross-core combine (the "all-reduce at the end") happens on
the host after gather.

Layout choices:
  - gates are computed as [batch=32 partitions, 2048 free] in PSUM, with the
    gate columns permuted to [i, f, o, g] so sigmoid runs as one [32,1536] op
    and tanh as one [32,512] op per layer.
  - the recurrent matmuls need h transposed ([hidden, batch] as lhsT); h is
    transposed each step on the PE (4x 32x128 transposes into one PSUM bank).
  - layer-0 input x is pre-transposed on the host into [86, T*32] SBUF rows:
    row 0 = raw z, rows 1..84 = covariates+embedding, row 85 = ones (bias0).
    Teacher forcing patches row 0 in place (copy_predicated with mu_{t-1})
    before the K=86 x-matmul of each step.
  - the Gaussian head is deinterleaved on the host: mu = h0.a + h1.b + bias,
    computed as 8 accumulating K=128, N=32 matmuls against h0T/h1T.
  - softplus/ln (different ACT table sets) are deferred to one end-of-kernel
    batch over all 96 steps so the recurrence only ever uses sigmoid/tanh
    (same table set -> zero table reloads in the hot loop).
"""

import os
import sys

import numpy as np

for _p in ("/opt/trn_rl_repo", "/root/.axon_site/_ro/trn_rl_repo"):
    if os.path.isdir(_p) and _p not in sys.path:
        sys.path.insert(0, _p)

import concourse.bass as bass
import concourse.mybir as mybir
from concourse import bacc
from concourse.masks import make_identity
from concourse.tile import TileContext

AF = mybir.ActivationFunctionType
OP = mybir.AluOpType
F32 = mybir.dt.float32
F32R = mybir.dt.float32r

T, B, H, NCORES = 96, 256, 512, 8
BC = B // NCORES          # 32 batch rows per core
XR = 86                   # z + 20 covariates + 64 embedding + ones
G = 4 * H                 # 2048 gate columns
KT = H // 128             # 4 k-tiles of the hidden dim
# gate columns permuted to [i, f, o, g]
PERM = np.concatenate([np.r_[0:512], np.r_[512:1024], np.r_[1536:2048], np.r_[1024:1536]])
LOG2PI = float(np.log(2.0 * np.pi))


def build_nc(t_steps=T, mm_f32r=False):
    nc = bacc.Bacc(None, target_bir_lowering=False)
    TS = t_steps
    N32 = TS * BC

    xt_d = nc.declare_dram_parameter("xt", [XR, N32], F32, isOutput=False)
    labs_d = nc.declare_dram_parameter("labs", [1, N32], F32, isOutput=False)
    whh0_d = nc.declare_dram_parameter("whh0", [128, KT, G], F32, isOutput=False)
    whh1_d = nc.declare_dram_parameter("whh1", [128, KT, G], F32, isOutput=False)
    wih1_d = nc.declare_dram_parameter("wih1", [128, KT, G], F32, isOutput=False)
    wxz_d = nc.declare_dram_parameter("wxz", [XR, G], F32, isOutput=False)
    b1_d = nc.declare_dram_parameter("b1", [1, G], F32, isOutput=False)
    hwmu_d = nc.declare_dram_parameter("hwmu", [128, 2 * KT], F32, isOutput=False)
    hwps_d = nc.declare_dram_parameter("hwps", [128, 2 * KT], F32, isOutput=False)
    hbmu_d = nc.declare_dram_parameter("hbmu", [1, 1], F32, isOutput=False)
    hbps_d = nc.declare_dram_parameter("hbps", [1, 1], F32, isOutput=False)
    hc_d = nc.declare_dram_parameter("hc", [128, H], F32, isOutput=False)
    ones_d = nc.declare_dram_parameter("ones1", [1, BC], F32, isOutput=False)

    ohc_d = nc.declare_dram_parameter("out_hc", [128, H], F32, isOutput=True)
    oms_d = nc.declare_dram_parameter("out_musig", [2, BC], F32, isOutput=True)
    olp_d = nc.declare_dram_parameter("out_loss", [2, TS], F32, isOutput=True)

    # matmul-operand dtype: float32r streams 1 col/cycle (vs 4 for float32);
    # every producer of these tiles must then also write float32r.
    MDT = F32R if mm_f32r else F32
    mm = lambda ap: ap

    with TileContext(nc) as tc:
        with (
            tc.tile_pool(name="wp", bufs=1) as wp,
            tc.tile_pool(name="st", bufs=1) as st,
            tc.tile_pool(name="wk", bufs=2) as wk,
            tc.tile_pool(name="fin", bufs=2) as fp,
            tc.tile_pool(name="ps", bufs=2, space="PSUM") as ps,
        ):
            # ---- resident inputs ----
            xt = wp.tile_from(xt_d[:, :], dtype=MDT, name="xt_sb")
            whh0 = wp.tile_from(whh0_d[:, :, :], dtype=MDT, name="whh0_sb")
            whh1 = wp.tile_from(whh1_d[:, :, :], dtype=MDT, name="whh1_sb")
            wih1 = wp.tile_from(wih1_d[:, :, :], dtype=MDT, name="wih1_sb")
            wxz = wp.tile_from(wxz_d[:, :], dtype=MDT, name="wxz_sb")
            b1 = wp.tile_from(b1_d[:, :], dtype=MDT, name="b1_sb")
            hwmu = wp.tile_from(hwmu_d[:, :], dtype=MDT, name="hwmu_sb")
            hwps = wp.tile_from(hwps_d[:, :], dtype=MDT, name="hwps_sb")
            hbmu = wp.tile_from(hbmu_d[:, :], name="hbmu_sb")
            hbps = wp.tile_from(hbps_d[:, :], name="hbps_sb")

            ident = wp.tile([BC, BC], F32, name="ident")
            make_identity(nc, ident)
            ones1 = wp.tile_from(ones_d[:, :], dtype=MDT, name="ones1_sb")

            # ---- persistent state ----
            mubuf = st.tile([1, N32], F32, name="mubuf")
            psbuf = st.tile([1, N32], F32, name="psbuf")
            ssum = st.tile([1, TS], F32, name="ssum")
            cnt = st.tile([1, TS], F32, name="cnt")
            c0 = st.tile([BC, H], F32, name="c0")
            c1 = st.tile([BC, H], F32, name="c1")
            nc.sync.dma_start(out=c0, in_=hc_d[64:96, :])
            nc.sync.dma_start(out=c1, in_=hc_d[96:128, :])

            h0 = wk.tile([BC, H], F32, name="h0", tag="h0")
            h1 = wk.tile([BC, H], F32, name="h1", tag="h1")
            nc.sync.dma_start(out=h0, in_=hc_d[0:32, :])
            nc.sync.dma_start(out=h1, in_=hc_d[32:64, :])

            def transpose_h(h, label):
                tx = ps.tile([128, KT * BC], F32, tag="small", name=f"tx{label}")
                for k in range(KT):
                    nc.tensor.transpose(
                        tx[:, k * BC:(k + 1) * BC], h[:, k * 128:(k + 1) * 128], ident
                    )
                hT = wk.tile([128, KT * BC], MDT, tag=f"hT{label[0]}", name=f"hT{label}")
                nc.vector.tensor_copy(hT, tx)
                return hT

            h0T = transpose_h(h0, "0i")
            h1T = transpose_h(h1, "1i")

            IFO, GSL = slice(0, 1536), slice(1536, 2048)

            def gate_matmuls(gifo, gg, lhsT, w, start, stop):
                """accumulate lhsT.T @ w[k] into [gifo | gg] for 4 k-tiles"""
                for k in range(KT):
                    lhs = mm(lhsT[:, k * BC:(k + 1) * BC])
                    st_ = start and k == 0
                    sp_ = stop and k == KT - 1
                    for j in range(3):
                        nc.tensor.matmul(
                            gifo[:, j * 512:(j + 1) * 512], lhs,
                            mm(w[:, k, j * 512:(j + 1) * 512]),
                            start=st_, stop=sp_,
                        )
                    nc.tensor.matmul(gg, lhs, mm(w[:, k, GSL]), start=st_, stop=sp_)

            def rank1_matmuls(gifo, gg, lhs_row, w_row, start, stop):
                for j in range(3):
                    nc.tensor.matmul(
                        gifo[:, j * 512:(j + 1) * 512], mm(lhs_row),
                        mm(w_row[:, j * 512:(j + 1) * 512]), start=start, stop=stop,
                    )
                nc.tensor.matmul(gg, mm(lhs_row), mm(w_row[:, GSL]), start=start, stop=stop)

            def cell_update(gifo, gg, sifo, tg, tcell, c, label):
                """sigmoid/tanh + c/h update; returns new h tile"""
                nc.scalar.activation(sifo, gifo, AF.Sigmoid)
                nc.scalar.activation(tg, gg, AF.Tanh)
                nc.vector.tensor_tensor(tg, sifo[:, 0:512], tg, OP.mult)        # i*g
                nc.vector.tensor_tensor(c, sifo[:, 512:1024], c, OP.mult)       # f*c
                nc.vector.tensor_tensor(c, c, tg, OP.add)
                nc.scalar.activation(tcell, c, AF.Tanh)
                h_new = wk.tile([BC, H], F32, tag=f"h{label}", name=f"h{label}_t")
                nc.vector.tensor_tensor(h_new, sifo[:, 1024:1536], tcell, OP.mult)
                return h_new

            for t in range(TS):
                tcols = slice(t * BC, (t + 1) * BC)

                # ---- gates layer 0: Whh0 @ h0T (+ x/z via K=86 matmul) ----
                gifo0 = ps.tile([BC, 1536], F32, tag="gifo", name="gifo0")
                gg0 = ps.tile([BC, 512], F32, tag="small", name="gg0")
                gate_matmuls(gifo0, gg0, h0T, whh0, start=True, stop=False)

                if t > 0:
                    # z_eff = z + (z==0)*(mu_prev - z), written in place on xt row 0
                    zrow = xt[0:1, tcols]
                    msk = wk.tile([1, BC], F32, tag="msk", name="msk")
                    nc.vector.tensor_scalar(msk, zrow, 0.0, None, OP.is_equal)
                    dlt = wk.tile([1, BC], F32, tag="dlt", name="dlt")
                    nc.vector.tensor_tensor(
                        dlt, mubuf[0:1, (t - 1) * BC:t * BC], zrow, OP.subtract
                    )
                    nc.vector.tensor_tensor(dlt, dlt, msk, OP.mult)
                    nc.vector.tensor_tensor(zrow, zrow, dlt, OP.add)
                xl = xt[:, tcols]
                for j in range(3):
                    nc.tensor.matmul(
                        gifo0[:, j * 512:(j + 1) * 512], mm(xl),
                        mm(wxz[:, j * 512:(j + 1) * 512]), start=False, stop=True,
                    )
                nc.tensor.matmul(gg0, mm(xl), mm(wxz[:, GSL]), start=False, stop=True)

                # ---- gates layer 1 (part A): Whh1 @ h1T + bias1 ----
                gifo1 = ps.tile([BC, 1536], F32, tag="gifo", name="gifo1")
                gg1 = ps.tile([BC, 512], F32, tag="small", name="gg1")
                gate_matmuls(gifo1, gg1, h1T, whh1, start=True, stop=False)
                rank1_matmuls(gifo1, gg1, ones1, b1, start=False, stop=False)

                # ---- layer 0 activations & state ----
                sifo0 = wk.tile([BC, 1536], F32, tag="sifo", name="sifo0")
                tg0 = wk.tile([BC, 512], F32, tag="tg", name="tg0")
                tc0 = wk.tile([BC, 512], F32, tag="tc", name="tc0")
                h0 = cell_update(gifo0, gg0, sifo0, tg0, tc0, c0, "0")
                h0T = transpose_h(h0, f"0_{t}")

                # ---- gates layer 1 (part B): Wih1 @ h0T ----
                gate_matmuls(gifo1, gg1, h0T, wih1, start=False, stop=True)

                # ---- layer 1 activations & state ----
                sifo1 = wk.tile([BC, 1536], F32, tag="sifo", name="sifo1")
                tg1 = wk.tile([BC, 512], F32, tag="tg", name="tg1")
                tc1 = wk.tile([BC, 512], F32, tag="tc", name="tc1")
                h1 = cell_update(gifo1, gg1, sifo1, tg1, tc1, c1, "1")
                h1T = transpose_h(h1, f"1_{t}")

                # ---- head: mu/ps rows via 8 accumulating K=128,N=32 matmuls ----
                hdmu = ps.tile([1, BC], F32, tag="small", name="hdmu")
                hdps = ps.tile([1, BC], F32, tag="small", name="hdps")
                for j in range(2 * KT):
                    hT = h0T if j < KT else h1T
                    rhs = hT[:, (j % KT) * BC:(j % KT + 1) * BC]
                    nc.tensor.matmul(
                        hdmu, mm(hwmu[:, j:j + 1]), mm(rhs),
                        start=(j == 0), stop=(j == 2 * KT - 1),
                    )
                    nc.tensor.matmul(
                        hdps, mm(hwps[:, j:j + 1]), mm(rhs),
                        start=(j == 0), stop=(j == 2 * KT - 1),
                    )
                nc.vector.tensor_scalar(mubuf[0:1, tcols], hdmu, hbmu, None, OP.add)
                nc.vector.tensor_scalar(psbuf[0:1, tcols], hdps, hbps, None, OP.add)

            # ---- deferred loss math (softplus/ln batched; 1 table swap each) ----
            NCH = 8 if TS % 8 == 0 else 1
            SPC = TS // NCH          # steps per chunk
            W = SPC * BC
            sig_last = None
            for ci in range(NCH):
                cc = slice(ci * W, (ci + 1) * W)
                # sigma = softplus(ps) = ln(1 + exp(ps));  Exp and Ln share a table set
                sig = fp.tile([1, W], F32, tag="fsig", name="sig")
                t1 = fp.tile([1, W], F32, tag="ft1", name="t1")
                nc.scalar.activation(t1, psbuf[0:1, cc], AF.Exp)
                nc.vector.tensor_scalar(t1, t1, 1.0, None, OP.add)
                nc.scalar.activation(sig, t1, AF.Ln)
                nc.vector.reciprocal(t1, sig)
                lb = fp.tile([1, W], F32, tag="flb", name="lb")
                nc.sync.dma_start(out=lb, in_=labs_d[0:1, cc])
                t2 = fp.tile([1, W], F32, tag="ft2", name="t2")
                nc.vector.tensor_tensor(t2, lb, mubuf[0:1, cc], OP.subtract)
                nc.vector.tensor_tensor(t2, t2, t1, OP.mult)
                nc.vector.tensor_tensor(t2, t2, t2, OP.mult)          # ((lab-mu)/sig)^2
                t3 = fp.tile([1, W], F32, tag="ft3", name="t3")
                nc.scalar.activation(t3, sig, AF.Ln)
                nc.vector.scalar_tensor_tensor(t2, t2, -0.5, t3, OP.mult, OP.subtract)
                nc.vector.tensor_scalar(t3, lb, 0.0, None, OP.not_equal)  # mask
                nc.vector.scalar_tensor_tensor(t2, t2, -0.5 * LOG2PI, t3, OP.add, OP.mult)
                csl = slice(ci * SPC, (ci + 1) * SPC)
                nc.vector.tensor_reduce(
                    ssum[0:1, csl], t2.rearrange("p (t b) -> p t b", b=BC),
                    mybir.AxisListType.X, OP.add,
                )
                nc.vector.tensor_reduce(
                    cnt[0:1, csl], t3.rearrange("p (t b) -> p t b", b=BC),
                    mybir.AxisListType.X, OP.add,
                )
                sig_last = sig

            # ---- outputs ----
            nc.sync.dma_start(out=ohc_d[0:32, :], in_=h0)
            nc.sync.dma_start(out=ohc_d[32:64, :], in_=h1)
            nc.sync.dma_start(out=ohc_d[64:96, :], in_=c0)
            nc.sync.dma_start(out=ohc_d[96:128, :], in_=c1)
            nc.sync.dma_start(out=oms_d[0:1, :], in_=mubuf[0:1, (TS - 1) * BC:TS * BC])
            nc.sync.dma_start(out=oms_d[1:2, :], in_=sig_last[0:1, (SPC - 1) * BC:SPC * BC])
            nc.sync.dma_start(out=olp_d[0:1, :], in_=ssum)
            nc.sync.dma_start(out=olp_d[1:2, :], in_=cnt)

    nc.compile()
    return nc


# ------------------------- host side -------------------------

def _pack_w(Wt_perm):
    """[512, 2048] (already col-permuted W.T) -> [128, 4, 2048] k-tile major"""
    return np.ascontiguousarray(
        Wt_perm.reshape(KT, 128, G).transpose(1, 0, 2), np.float32
    )


def host_prep(inputs, t_steps=T):
    f = lambda k: np.asarray(inputs[k], np.float32)
    train_batch, labels = f("train_batch"), f("labels_batch")
    hidden, cell, embed = f("hidden"), f("cell"), f("embed")
    W_ih0, W_hh0 = f("W_ih0"), f("W_hh0")
    W_ih1, W_hh1 = f("W_ih1"), f("W_hh1")
    b_ih0, b_hh0 = f("b_ih0"), f("b_hh0")
    b_ih1, b_hh1 = f("b_ih1"), f("b_hh1")
    W_mu, b_mu, W_ps, b_ps = f("W_mu"), f("b_mu"), f("W_ps"), f("b_ps")
    idx = np.asarray(inputs["idx"])
    TS = t_steps

    emb = embed[idx[0]]                                    # [B, 64]
    shared = {
        "whh0": _pack_w(np.ascontiguousarray(W_hh0.T[:, PERM])),
        "whh1": _pack_w(np.ascontiguousarray(W_hh1.T[:, PERM])),
        "wih1": _pack_w(np.ascontiguousarray(W_ih1.T[:, PERM])),
        "wxz": np.ascontiguousarray(
            np.concatenate([W_ih0.T[:, PERM], (b_ih0 + b_hh0)[None, PERM]], 0),
            np.float32),
        "b1": np.ascontiguousarray((b_ih1 + b_hh1)[None, PERM], np.float32),
        "hwmu": np.ascontiguousarray(np.concatenate(
            [W_mu[0, 0::2].reshape(KT, 128).T, W_mu[0, 1::2].reshape(KT, 128).T], 1)),
        "hwps": np.ascontiguousarray(np.concatenate(
            [W_ps[0, 0::2].reshape(KT, 128).T, W_ps[0, 1::2].reshape(KT, 128).T], 1)),
        "hbmu": b_mu.reshape(1, 1),
        "hbps": b_ps.reshape(1, 1),
    }

    in_maps = []
    for core in range(NCORES):
        sl = slice(core * BC, (core + 1) * BC)
        xc = train_batch[:TS, sl, :]                       # [TS, 32, 21]
        xt = np.empty((XR, TS, BC), np.float32)
        xt[0] = xc[:, :, 0]
        xt[1:21] = xc[:, :, 1:].transpose(2, 0, 1)
        xt[21:85] = np.broadcast_to(emb[sl].T[:, None, :], (64, TS, BC))
        xt[85] = 1.0
        m = dict(shared)
        m["ones1"] = np.ones((1, BC), np.float32)
        m["xt"] = np.ascontiguousarray(xt.reshape(XR, TS * BC))
        m["labs"] = np.ascontiguousarray(labels[:TS, sl].reshape(1, TS * BC))
        m["hc"] = np.ascontiguousarray(np.concatenate(
            [hidden[0, sl], hidden[1, sl], cell[0, sl], cell[1, sl]], 0))
        in_maps.append(m)
    return in_maps


def assemble(results, t_steps=T):
    mus = np.concatenate([r["out_musig"][0] for r in results]).astype(np.float32)
    sigmas = np.concatenate([r["out_musig"][1] for r in results]).astype(np.float32)
    h = np.stack([
        np.concatenate([r["out_hc"][0:32] for r in results], 0),
        np.concatenate([r["out_hc"][32:64] for r in results], 0),
    ]).astype(np.float32)
    c = np.stack([
        np.concatenate([r["out_hc"][64:96] for r in results], 0),
        np.concatenate([r["out_hc"][96:128] for r in results], 0),
    ]).astype(np.float32)
    S = np.sum([r["out_loss"][0] for r in results], axis=0, dtype=np.float32)
    CNT = np.sum([r["out_loss"][1] for r in results], axis=0, dtype=np.float32)
    n = np.maximum(CNT, 1.0).astype(np.float32)
    loss = np.float32(np.sum(-(S / n), dtype=np.float32))
    return mus, sigmas, h, c, np.array([loss], np.float32)


_CACHE = {}
LAST_EXEC_NS = None


def kernel(**inputs):
    global LAST_EXEC_NS
    from concourse.bass_utils import run_bass_kernel_spmd

    mm_f32r = os.environ.get("LSTM_MM_F32R", "0") == "1"
    trace = os.environ.get("LSTM_TRACE", "0") == "1"
    key = (T, mm_f32r)
    if key not in _CACHE:
        _CACHE[key] = build_nc(T, mm_f32r=mm_f32r)
    nc = _CACHE[key]
    in_maps = host_prep(inputs, T)
    res = run_bass_kernel_spmd(nc, in_maps, core_ids=list(range(NCORES)), trace=trace)
    LAST_EXEC_NS = res.exec_time_ns
    return assemble(res.results, T)


# revision 16
# speedup vs baseline: 3.5412x; 1.1975x over previous
"""Trainium2 Bass kernel for a 2-layer DeepAR-style LSTM (T=96, B=256, H=512).

Strategy: 8-way data parallel over batch (32 rows/core), zero in-kernel
collectives.  Each core runs the full 96-step recurrence with all weights
resident in SBUF.  Per-step loss terms are emitted as per-step partial
sums/counts; the cross-core combine (the "all-reduce at the end") happens on
the host after gather.

Layout choices:
  - gates are computed as [batch=32 partitions, 2048 free] in PSUM with gate
    columns permuted to [i, f, o, g], split into an A tile [i|f] (on the
    critical path: sigmoid(i,f) feeds the cell update) and a B tile [o|g].
  - matmuls run in float32r (1 col/cycle on the PE vs 4 for float32).
  - the recurrent matmuls need h transposed; h is transposed each step on the
    PE (4x 32x128 transposes into one PSUM bank) then copied to SBUF.
  - layer-0 input x is pre-transposed on the host into [86, T*32] SBUF rows:
    row 0 = raw z, rows 1..84 = covariates+embedding, row 85 = ones (bias0).
    Teacher forcing: z_eff = z + (z==0)*mu_{t-1}; the z==0 mask for all steps
    is precomputed once, so the per-step patch is 2 tiny DVE ops in place.
  - the PE stream is software-pipelined: Whh1(t)+bias1(t) fill the PE during
    layer-0 activations, Whh0(t+1) fills it during layer-1 activations, so
    the PE never idles long enough for the HAM clock gate to re-throttle.
  - the Gaussian head is deinterleaved on the host; one [2,32] PSUM tile via
    8 accumulating K=128,N=32 matmuls; the ps row is DMA'd to DRAM and its
    bias is folded into the finalize Exp activation bias.
  - softplus/ln (a different ACT table set than sigmoid/tanh) are deferred to
    one end-of-kernel batch so the hot loop never swaps ACT tables.
"""

import os
import sys

import numpy as np

for _p in ("/opt/trn_rl_repo", "/root/.axon_site/_ro/trn_rl_repo"):
    if os.path.isdir(_p) and _p not in sys.path:
        sys.path.insert(0, _p)

import concourse.bass as bass
import concourse.mybir as mybir
from concourse import bacc
from concourse.masks import make_identity
from concourse.tile import TileContext

AF = mybir.ActivationFunctionType
OP = mybir.AluOpType
F32 = mybir.dt.float32
F32R = mybir.dt.float32r

T, B, H, NCORES = 96, 256, 512, 8
BC = B // NCORES          # 32 batch rows per core
XR = 86                   # z + 20 covariates + 64 embedding + ones
G = 4 * H                 # 2048 gate columns
KT = H // 128             # 4 k-tiles of the hidden dim
# gate columns permuted to [i, f, o, g]
PERM = np.concatenate([np.r_[0:512], np.r_[512:1024], np.r_[1536:2048], np.r_[1024:1536]])
LOG2PI = float(np.log(2.0 * np.pi))


def build_nc(t_steps=T, mm_f32r=True):
    nc = bacc.Bacc(None, target_bir_lowering=False)
    TS = t_steps
    N32 = TS * BC

    xt_d = nc.declare_dram_parameter("xt", [XR, N32], F32, isOutput=False)
    labs_d = nc.declare_dram_parameter("labs", [1, N32], F32, isOutput=False)
    whh0_d = nc.declare_dram_parameter("whh0", [128, KT, G], F32, isOutput=False)
    whh1_d = nc.declare_dram_parameter("whh1", [128, KT, G], F32, isOutput=False)
    wih1_d = nc.declare_dram_parameter("wih1", [128, KT, G], F32, isOutput=False)
    wxz_d = nc.declare_dram_parameter("wxz", [XR, G], F32, isOutput=False)
    b1_d = nc.declare_dram_parameter("b1", [1, G], F32, isOutput=False)
    hwc_d = nc.declare_dram_parameter("hwc", [128, 2 * KT, 2], F32, isOutput=False)
    hbmu_d = nc.declare_dram_parameter("hbmu", [1, 1], F32, isOutput=False)
    hbps_d = nc.declare_dram_parameter("hbps", [1, 1], F32, isOutput=False)
    hc_d = nc.declare_dram_parameter("hc", [128, H], F32, isOutput=False)
    ones_d = nc.declare_dram_parameter("ones1", [1, BC], F32, isOutput=False)

    ohc_d = nc.declare_dram_parameter("out_hc", [128, H], F32, isOutput=True)
    oms_d = nc.declare_dram_parameter("out_musig", [2, BC], F32, isOutput=True)
    olp_d = nc.declare_dram_parameter("out_loss", [2, TS], F32, isOutput=True)

    psraw_d = nc.dram_tensor("psraw", [1, N32], F32)

    MDT = F32R if mm_f32r else F32

    with TileContext(nc) as tc:
        with (
            tc.tile_pool(name="wp", bufs=1) as wp,
            tc.tile_pool(name="st", bufs=1) as st,
            tc.tile_pool(name="wk", bufs=2) as wk,
            tc.tile_pool(name="fin", bufs=2) as fp,
            tc.tile_pool(name="ps", bufs=2, space="PSUM") as ps,
        ):
            # ---- resident inputs ----
            xt = wp.tile_from(xt_d[:, :], dtype=MDT, name="xt_sb")
            whh0 = wp.tile_from(whh0_d[:, :, :], dtype=MDT, name="whh0_sb")
            whh1 = wp.tile_from(whh1_d[:, :, :], dtype=MDT, name="whh1_sb")
            wih1 = wp.tile_from(wih1_d[:, :, :], dtype=MDT, name="wih1_sb")
            wxz = wp.tile_from(wxz_d[:, :], dtype=MDT, name="wxz_sb")
            b1 = wp.tile_from(b1_d[:, :], dtype=MDT, name="b1_sb")
            hwc = wp.tile_from(hwc_d[:, :, :], dtype=MDT, name="hwc_sb")
            hbmu = wp.tile_from(hbmu_d[:, :], name="hbmu_sb")
            hbps = wp.tile_from(hbps_d[:, :], name="hbps_sb")
            ident = wp.tile([BC, BC], F32, name="ident")
            make_identity(nc, ident)
            ones1 = wp.tile_from(ones_d[:, :], dtype=MDT, name="ones1_sb")

            # ---- persistent state ----
            mubuf = st.tile([1, N32], F32, name="mubuf")
            iz = st.tile([1, N32], F32, name="iz")
            nc.vector.tensor_scalar(iz, xt[0:1, :], 0.0, None, OP.is_equal)
            ssum = st.tile([1, TS], F32, name="ssum")
            cnt = st.tile([1, TS], F32, name="cnt")
            c0 = st.tile([BC, H], F32, name="c0")
            c1 = st.tile([BC, H], F32, name="c1")
            nc.sync.dma_start(out=c0, in_=hc_d[64:96, :])
            nc.sync.dma_start(out=c1, in_=hc_d[96:128, :])
            h0 = wk.tile([BC, H], F32, name="h0", tag="h0")
            h1 = wk.tile([BC, H], F32, name="h1", tag="h1")
            nc.sync.dma_start(out=h0, in_=hc_d[0:32, :])
            nc.sync.dma_start(out=h1, in_=hc_d[32:64, :])

            def transpose_h(h, label):
                tx = ps.tile([BC, 1024], F32, tag="gb", name=f"tx{label}",
                             padded_shape=[128, 1024])
                txv = tx.tensor[0:128, 0:KT * BC]
                for k in range(KT):
                    nc.tensor.transpose(
                        txv[:, k * BC:(k + 1) * BC], h[:, k * 128:(k + 1) * 128], ident
                    )
                hT = wk.tile([128, KT * BC], MDT, tag=f"hT{label[0]}", name=f"hT{label}")
                nc.vector.tensor_copy(hT, txv)
                return hT

            def gate_block(ga, gb, lhsT, w, start, stop):
                """accumulate lhsT.T @ w[k] into [ga | gb]; A chunks first"""
                for cbase, tile in ((0, ga), (2, gb)):
                    for k in range(KT):
                        lhs = lhsT[:, k * BC:(k + 1) * BC]
                        st_ = start and k == 0
                        sp_ = stop and k == KT - 1
                        for cj in range(2):
                            c = cbase + cj
                            nc.tensor.matmul(
                                tile[:, cj * 512:(cj + 1) * 512], lhs,
                                w[:, k, c * 512:(c + 1) * 512], start=st_, stop=sp_,
                            )

            def rank1_block(ga, gb, lhs_row, w_row):
                for cbase, tile in ((0, ga), (2, gb)):
                    for cj in range(2):
                        c = cbase + cj
                        nc.tensor.matmul(
                            tile[:, cj * 512:(cj + 1) * 512], lhs_row,
                            w_row[:, c * 512:(c + 1) * 512], start=False, stop=False,
                        )

            def xz_block(ga, gb, tcols):
                xl = xt[:, tcols]
                for cbase, tile in ((0, ga), (2, gb)):
                    for cj in range(2):
                        c = cbase + cj
                        nc.tensor.matmul(
                            tile[:, cj * 512:(cj + 1) * 512], xl,
                            wxz[:, c * 512:(c + 1) * 512], start=False, stop=True,
                        )

            def acts(ga, gb, c, hname):
                """sigmoid/tanh + cell update; returns new h tile"""
                sif = wk.tile([BC, 1024], F32, tag="sif", name=f"sif{hname}")
                nc.scalar.activation(sif, ga, AF.Sigmoid)
                tg = wk.tile([BC, 512], F32, tag="tg", name=f"tg{hname}")
                nc.scalar.activation(tg, gb[:, 512:1024], AF.Tanh)
                so = wk.tile([BC, 512], F32, tag="so", name=f"so{hname}")
                nc.scalar.activation(so, gb[:, 0:512], AF.Sigmoid)
                nc.vector.tensor_tensor(tg, sif[:, 0:512], tg, OP.mult)   # i*g
                nc.vector.tensor_tensor(c, sif[:, 512:1024], c, OP.mult)  # f*c
                nc.vector.tensor_tensor(c, c, tg, OP.add)
                tcell = wk.tile([BC, 512], F32, tag="tc", name=f"tc{hname}")
                nc.scalar.activation(tcell, c, AF.Tanh)
                h_new = wk.tile([BC, H], F32, tag=f"h{hname}", name=f"h{hname}_t")
                nc.vector.tensor_tensor(h_new, so, tcell, OP.mult)
                return h_new

            # ---- preamble: initial transposes + G0(0) ----
            h0T = transpose_h(h0, "0i")
            h1T = transpose_h(h1, "1i")
            ga0 = ps.tile([BC, 1024], F32, tag="ga", name="ga0_p")
            gb0 = ps.tile([BC, 1024], F32, tag="gb", name="gb0_p")
            gate_block(ga0, gb0, h0T, whh0, start=True, stop=False)

            for t in range(TS):
                tcols = slice(t * BC, (t + 1) * BC)

                # teacher forcing: z += (z==0)*mu_prev, in place on xt row 0
                if t > 0:
                    zrow = xt[0:1, tcols]
                    dlt = wk.tile([1, BC], F32, tag="dlt", name="dlt")
                    nc.vector.tensor_tensor(
                        dlt, mubuf[0:1, (t - 1) * BC:t * BC], iz[0:1, tcols], OP.mult
                    )
                    nc.vector.tensor_tensor(zrow, zrow, dlt, OP.add)
                xz_block(ga0, gb0, tcols)                      # G0(t) complete

                # G1(t) part A: Whh1 @ h1T(t-1) + bias1 (fills PE during acts0)
                ga1 = ps.tile([BC, 1024], F32, tag="ga", name="ga1")
                gb1 = ps.tile([BC, 1024], F32, tag="gb", name="gb1")
                gate_block(ga1, gb1, h1T, whh1, start=True, stop=False)
                rank1_block(ga1, gb1, ones1, b1)

                h0 = acts(ga0, gb0, c0, "0")
                h0T = transpose_h(h0, f"0_{t}")

                gate_block(ga1, gb1, h0T, wih1, start=False, stop=True)  # G1 done

                # G0(t+1): Whh0 @ h0T(t) (fills PE during acts1)
                if t + 1 < TS:
                    ga0 = ps.tile([BC, 1024], F32, tag="ga", name="ga0")
                    gb0 = ps.tile([BC, 1024], F32, tag="gb", name="gb0")
                    gate_block(ga0, gb0, h0T, whh0, start=True, stop=False)

                h1 = acts(ga1, gb1, c1, "1")
                h1T = transpose_h(h1, f"1_{t}")

                # head: [mu; ps] = 8 accumulating K=128,N=32 matmuls
                hd = ps.tile([2, BC], F32, tag="ga", name="hd",
                             padded_shape=[BC, 1024])
                for j in range(2 * KT):
                    hT = h0T if j < KT else h1T
                    rhs = hT[:, (j % KT) * BC:(j % KT + 1) * BC]
                    nc.tensor.matmul(
                        hd, hwc[:, j, :], rhs,
                        start=(j == 0), stop=(j == 2 * KT - 1),
                    )
                hdsb = wk.tile([2, BC], F32, tag="hdsb", name="hdsb")
                nc.vector.tensor_copy(hdsb, hd)
                nc.vector.tensor_scalar(mubuf[0:1, tcols], hdsb[0:1, :], hbmu, None, OP.add)
                nc.sync.dma_start(out=psraw_d[0:1, tcols], in_=hdsb[1:2, :])

            # ---- deferred loss math (Exp/Ln share one ACT table set) ----
            NCH = 8 if TS % 8 == 0 else 1
            SPC = TS // NCH          # steps per chunk
            W = SPC * BC
            sig_last = None
            for ci in range(NCH):
                cc = slice(ci * W, (ci + 1) * W)
                pb = fp.tile([1, W], F32, tag="fpb", name="pb")
                nc.sync.dma_start(out=pb, in_=psraw_d[0:1, cc])
                sig = fp.tile([1, W], F32, tag="fsig", name="sig")
                t1 = fp.tile([1, W], F32, tag="ft1", name="t1")
                # sigma = softplus(ps + b_ps) = ln(1 + exp(ps + b_ps))
                nc.scalar.activation(t1, pb, AF.Exp, bias=hbps[0:1, 0:1])
                nc.vector.tensor_scalar(t1, t1, 1.0, None, OP.add)
                nc.scalar.activation(sig, t1, AF.Ln)
                nc.vector.reciprocal(t1, sig)
                lb = fp.tile([1, W], F32, tag="flb", name="lb")
                nc.sync.dma_start(out=lb, in_=labs_d[0:1, cc])
                t2 = fp.tile([1, W], F32, tag="ft2", name="t2")
                nc.vector.tensor_tensor(t2, lb, mubuf[0:1, cc], OP.subtract)
                nc.vector.tensor_tensor(t2, t2, t1, OP.mult)
                nc.vector.tensor_tensor(t2, t2, t2, OP.mult)          # ((lab-mu)/sig)^2
                t3 = fp.tile([1, W], F32, tag="ft3", name="t3")
                nc.scalar.activation(t3, sig, AF.Ln)
                nc.vector.scalar_tensor_tensor(t2, t2, -0.5, t3, OP.mult, OP.subtract)
                nc.vector.tensor_scalar(t3, lb, 0.0, None, OP.not_equal)  # mask
                nc.vector.scalar_tensor_tensor(t2, t2, -0.5 * LOG2PI, t3, OP.add, OP.mult)
                csl = slice(ci * SPC, (ci + 1) * SPC)
                nc.vector.tensor_reduce(
                    ssum[0:1, csl], t2.rearrange("p (t b) -> p t b", b=BC),
                    mybir.AxisListType.X, OP.add,
                )
                nc.vector.tensor_reduce(
                    cnt[0:1, csl], t3.rearrange("p (t b) -> p t b", b=BC),
                    mybir.AxisListType.X, OP.add,
                )
                sig_last = sig

            # ---- outputs ----
            nc.sync.dma_start(out=ohc_d[0:32, :], in_=h0)
            nc.sync.dma_start(out=ohc_d[32:64, :], in_=h1)
            nc.sync.dma_start(out=ohc_d[64:96, :], in_=c0)
            nc.sync.dma_start(out=ohc_d[96:128, :], in_=c1)
            nc.sync.dma_start(out=oms_d[0:1, :], in_=mubuf[0:1, (TS - 1) * BC:TS * BC])
            nc.sync.dma_start(out=oms_d[1:2, :], in_=sig_last[0:1, (SPC - 1) * BC:SPC * BC])
            nc.sync.dma_start(out=olp_d[0:1, :], in_=ssum)
            nc.sync.dma_start(out=olp_d[1:2, :], in_=cnt)

    nc.compile()
    return nc


# ------------------------- host side -------------------------

def _pack_w(Wt_perm):
    """[512, 2048] (already col-permuted W.T) -> [128, 4, 2048] k-tile major"""
    return np.ascontiguousarray(
        Wt_perm.reshape(KT, 128, G).transpose(1, 0, 2), np.float32
    )


def host_prep(inputs, t_steps=T):
    f = lambda k: np.asarray(inputs[k], np.float32)
    train_batch, labels = f("train_batch"), f("labels_batch")
    hidden, cell, embed = f("hidden"), f("cell"), f("embed")
    W_ih0, W_hh0 = f("W_ih0"), f("W_hh0")
    W_ih1, W_hh1 = f("W_ih1"), f("W_hh1")
    b_ih0, b_hh0 = f("b_ih0"), f("b_hh0")
    b_ih1, b_hh1 = f("b_ih1"), f("b_hh1")
    W_mu, b_mu, W_ps, b_ps = f("W_mu"), f("b_mu"), f("W_ps"), f("b_ps")
    idx = np.asarray(inputs["idx"])
    TS = t_steps

    emb = embed[idx[0]]                                    # [B, 64]
    # head weights deinterleaved: [128, j, {mu,ps}]; j=0..3 h0 k-tiles, 4..7 h1
    hwc = np.zeros((128, 2 * KT, 2), np.float32)
    hwc[:, 0:KT, 0] = W_mu[0, 0::2].reshape(KT, 128).T
    hwc[:, KT:2 * KT, 0] = W_mu[0, 1::2].reshape(KT, 128).T
    hwc[:, 0:KT, 1] = W_ps[0, 0::2].reshape(KT, 128).T
    hwc[:, KT:2 * KT, 1] = W_ps[0, 1::2].reshape(KT, 128).T
    shared = {
        "whh0": _pack_w(np.ascontiguousarray(W_hh0.T[:, PERM])),
        "whh1": _pack_w(np.ascontiguousarray(W_hh1.T[:, PERM])),
        "wih1": _pack_w(np.ascontiguousarray(W_ih1.T[:, PERM])),
        "wxz": np.ascontiguousarray(
            np.concatenate([W_ih0.T[:, PERM], (b_ih0 + b_hh0)[None, PERM]], 0),
            np.float32),
        "b1": np.ascontiguousarray((b_ih1 + b_hh1)[None, PERM], np.float32),
        "hwc": hwc,
        "hbmu": b_mu.reshape(1, 1),
        "hbps": b_ps.reshape(1, 1),
    }

    in_maps = []
    for core in range(NCORES):
        sl = slice(core * BC, (core + 1) * BC)
        xc = train_batch[:TS, sl, :]                       # [TS, 32, 21]
        xt = np.empty((XR, TS, BC), np.float32)
        xt[0] = xc[:, :, 0]
        xt[1:21] = xc[:, :, 1:].transpose(2, 0, 1)
        xt[21:85] = np.broadcast_to(emb[sl].T[:, None, :], (64, TS, BC))
        xt[85] = 1.0
        m = dict(shared)
        m["ones1"] = np.ones((1, BC), np.float32)
        m["xt"] = np.ascontiguousarray(xt.reshape(XR, TS * BC))
        m["labs"] = np.ascontiguousarray(labels[:TS, sl].reshape(1, TS * BC))
        m["hc"] = np.ascontiguousarray(np.concatenate(
            [hidden[0, sl], hidden[1, sl], cell[0, sl], cell[1, sl]], 0))
        in_maps.append(m)
    return in_maps


def assemble(results, t_steps=T):
    mus = np.concatenate([r["out_musig"][0] for r in results]).astype(np.float32)
    sigmas = np.concatenate([r["out_musig"][1] for r in results]).astype(np.float32)
    h = np.stack([
        np.concatenate([r["out_hc"][0:32] for r in results], 0),
        np.concatenate([r["out_hc"][32:64] for r in results], 0),
    ]).astype(np.float32)
    c = np.stack([
        np.concatenate([r["out_hc"][64:96] for r in results], 0),
        np.concatenate([r["out_hc"][96:128] for r in results], 0),
    ]).astype(np.float32)
    S = np.sum([r["out_loss"][0] for r in results], axis=0, dtype=np.float32)
    CNT = np.sum([r["out_loss"][1] for r in results], axis=0, dtype=np.float32)
    n = np.maximum(CNT, 1.0).astype(np.float32)
    loss = np.float32(np.sum(-(S / n), dtype=np.float32))
    return mus, sigmas, h, c, np.array([loss], np.float32)


_CACHE = {}
LAST_EXEC_NS = None


def kernel(**inputs):
    global LAST_EXEC_NS
    from concourse.bass_utils import run_bass_kernel_spmd

    mm_f32r = os.environ.get("LSTM_MM_F32R", "1") == "1"
    trace = os.environ.get("LSTM_TRACE", "0") == "1"
    key = (T, mm_f32r)
    if key not in _CACHE:
        _CACHE[key] = build_nc(T, mm_f32r=mm_f32r)
    nc = _CACHE[key]
    in_maps = host_prep(inputs, T)
    res = run_bass_kernel_spmd(nc, in_maps, core_ids=list(range(NCORES)), trace=trace)
    LAST_EXEC_NS = res.exec_time_ns
    return assemble(res.results, T)


# revision 18
# speedup vs baseline: 3.5491x; 1.0022x over previous
"""Trainium2 Bass kernel for a 2-layer DeepAR-style LSTM (T=96, B=256, H=512).

Strategy: 8-way data parallel over batch (32 rows/core), zero in-kernel
collectives.  Each core runs the full 96-step recurrence with all weights
resident in SBUF.  Per-step loss terms are emitted as per-step partial
sums/counts; the cross-core combine (the "all-reduce at the end") happens on
the host after gather.

Layout choices:
  - gates are computed as [batch=32 partitions, 2048 free] in PSUM with gate
    columns permuted to [i, f, o, g], split into an A tile [i|f] (on the
    critical path: sigmoid(i,f) feeds the cell update) and a B tile [o|g].
  - matmuls run in float32r (1 col/cycle on the PE vs 4 for float32).
  - the recurrent matmuls need h transposed; h is transposed each step on the
    PE (4x 32x128 transposes into one PSUM bank) then copied to SBUF.
  - layer-0 input x is pre-transposed on the host into [86, T*32] SBUF rows:
    row 0 = raw z, rows 1..84 = covariates+embedding, row 85 = ones (bias0).
    Teacher forcing: z_eff = z + (z==0)*mu_{t-1}; the z==0 mask for all steps
    is precomputed once, so the per-step patch is 2 tiny DVE ops in place.
  - the PE stream is software-pipelined: Whh1(t)+bias1(t) fill the PE during
    layer-0 activations, Whh0(t+1) fills it during layer-1 activations, so
    the PE never idles long enough for the HAM clock gate to re-throttle.
  - the Gaussian head is deinterleaved on the host; one [2,32] PSUM tile via
    8 accumulating K=128,N=32 matmuls; the ps row is DMA'd to DRAM and its
    bias is folded into the finalize Exp activation bias.
  - softplus/ln (a different ACT table set than sigmoid/tanh) are deferred to
    one end-of-kernel batch so the hot loop never swaps ACT tables.
"""

import os
import sys

import numpy as np

for _p in ("/opt/trn_rl_repo", "/root/.axon_site/_ro/trn_rl_repo"):
    if os.path.isdir(_p) and _p not in sys.path:
        sys.path.insert(0, _p)

import concourse.bass as bass
import concourse.mybir as mybir
from concourse import bacc
from concourse.masks import make_identity
from concourse.tile import TileContext

AF = mybir.ActivationFunctionType
OP = mybir.AluOpType
F32 = mybir.dt.float32
F32R = mybir.dt.float32r

T, B, H, NCORES = 96, 256, 512, 8
BC = B // NCORES          # 32 batch rows per core
XR = 86                   # z + 20 covariates + 64 embedding + ones
G = 4 * H                 # 2048 gate columns
KT = H // 128             # 4 k-tiles of the hidden dim
# gate columns permuted to [i, f, o, g]
PERM = np.concatenate([np.r_[0:512], np.r_[512:1024], np.r_[1536:2048], np.r_[1024:1536]])
LOG2PI = float(np.log(2.0 * np.pi))


def build_nc(t_steps=T, mm_f32r=True):
    nc = bacc.Bacc(None, target_bir_lowering=False)
    TS = t_steps
    N32 = TS * BC

    xt_d = nc.declare_dram_parameter("xt", [XR, N32], F32, isOutput=False)
    labs_d = nc.declare_dram_parameter("labs", [1, N32], F32, isOutput=False)
    whh0_d = nc.declare_dram_parameter("whh0", [128, KT, G], F32, isOutput=False)
    whh1_d = nc.declare_dram_parameter("whh1", [128, KT, G], F32, isOutput=False)
    wih1_d = nc.declare_dram_parameter("wih1", [128, KT, G], F32, isOutput=False)
    wxz_d = nc.declare_dram_parameter("wxz", [XR, G], F32, isOutput=False)
    b1_d = nc.declare_dram_parameter("b1", [1, G], F32, isOutput=False)
    hwc_d = nc.declare_dram_parameter("hwc", [128, 2 * KT, 2], F32, isOutput=False)
    hbmu_d = nc.declare_dram_parameter("hbmu", [1, 1], F32, isOutput=False)
    hbps_d = nc.declare_dram_parameter("hbps", [1, 1], F32, isOutput=False)
    hc_d = nc.declare_dram_parameter("hc", [128, H], F32, isOutput=False)
    ones_d = nc.declare_dram_parameter("ones1", [1, BC], F32, isOutput=False)

    ohc_d = nc.declare_dram_parameter("out_hc", [128, H], F32, isOutput=True)
    oms_d = nc.declare_dram_parameter("out_musig", [2, BC], F32, isOutput=True)
    olp_d = nc.declare_dram_parameter("out_loss", [2, TS], F32, isOutput=True)

    psraw_d = nc.dram_tensor("psraw", [1, N32], F32)

    MDT = F32R if mm_f32r else F32

    with TileContext(nc) as tc:
        with (
            tc.tile_pool(name="wp", bufs=1) as wp,
            tc.tile_pool(name="st", bufs=1) as st,
            tc.tile_pool(name="wk", bufs=2) as wk,
            tc.tile_pool(name="fin", bufs=2) as fp,
            tc.tile_pool(name="ps", bufs=2, space="PSUM") as ps,
        ):
            # ---- state + first-needed weights DMA'd first ----
            c0 = st.tile([BC, H], F32, name="c0")
            c1 = st.tile([BC, H], F32, name="c1")
            nc.sync.dma_start(out=c0, in_=hc_d[64:96, :])
            nc.sync.dma_start(out=c1, in_=hc_d[96:128, :])
            h0 = wk.tile([BC, H], F32, name="h0", tag="h0")
            h1 = wk.tile([BC, H], F32, name="h1", tag="h1")
            nc.sync.dma_start(out=h0, in_=hc_d[0:32, :])
            nc.sync.dma_start(out=h1, in_=hc_d[32:64, :])
            whh0 = wp.tile_from(whh0_d[:, :, :], dtype=MDT, name="whh0_sb")
            xt = wp.tile_from(xt_d[:, :], dtype=MDT, name="xt_sb")
            wxz = wp.tile_from(wxz_d[:, :], dtype=MDT, name="wxz_sb")
            whh1 = wp.tile_from(whh1_d[:, :, :], dtype=MDT, name="whh1_sb")
            wih1 = wp.tile_from(wih1_d[:, :, :], dtype=MDT, name="wih1_sb")
            b1 = wp.tile_from(b1_d[:, :], dtype=MDT, name="b1_sb")
            hwc = wp.tile_from(hwc_d[:, :, :], dtype=MDT, name="hwc_sb")
            hbmu = wp.tile_from(hbmu_d[:, :], name="hbmu_sb")
            hbps = wp.tile_from(hbps_d[:, :], name="hbps_sb")
            ident = wp.tile([BC, BC], F32, name="ident")
            make_identity(nc, ident)
            ones1 = wp.tile_from(ones_d[:, :], dtype=MDT, name="ones1_sb")

            # ---- persistent state ----
            mubuf = st.tile([1, N32], F32, name="mubuf")
            iz = st.tile([1, N32], F32, name="iz")
            nc.vector.tensor_scalar(iz, xt[0:1, :], 0.0, None, OP.is_equal)
            ssum = st.tile([1, TS], F32, name="ssum")
            cnt = st.tile([1, TS], F32, name="cnt")

            def transpose_h(h, label):
                tx = ps.tile([BC, 1024], F32, tag="gb", name=f"tx{label}",
                             padded_shape=[128, 1024])
                txv = tx.tensor[0:128, 0:KT * BC]
                for k in range(KT):
                    nc.tensor.transpose(
                        txv[:, k * BC:(k + 1) * BC], h[:, k * 128:(k + 1) * 128], ident
                    )
                hT = wk.tile([128, KT * BC], MDT, tag=f"hT{label[0]}", name=f"hT{label}")
                nc.vector.tensor_copy(hT, txv)
                return hT

            def gate_block(ga, gb, lhsT, w, start, stop):
                """accumulate lhsT.T @ w[k] into [ga | gb]; A chunks first"""
                for cbase, tile in ((0, ga), (2, gb)):
                    for k in range(KT):
                        lhs = lhsT[:, k * BC:(k + 1) * BC]
                        st_ = start and k == 0
                        sp_ = stop and k == KT - 1
                        for cj in range(2):
                            c = cbase + cj
                            nc.tensor.matmul(
                                tile[:, cj * 512:(cj + 1) * 512], lhs,
                                w[:, k, c * 512:(c + 1) * 512], start=st_, stop=sp_,
                            )

            def rank1_block(ga, gb, lhs_row, w_row):
                for cbase, tile in ((0, ga), (2, gb)):
                    for cj in range(2):
                        c = cbase + cj
                        nc.tensor.matmul(
                            tile[:, cj * 512:(cj + 1) * 512], lhs_row,
                            w_row[:, c * 512:(c + 1) * 512], start=False, stop=False,
                        )

            def xz_block(ga, gb, tcols):
                xl = xt[:, tcols]
                for cbase, tile in ((0, ga), (2, gb)):
                    for cj in range(2):
                        c = cbase + cj
                        nc.tensor.matmul(
                            tile[:, cj * 512:(cj + 1) * 512], xl,
                            wxz[:, c * 512:(c + 1) * 512], start=False, stop=True,
                        )

            def acts(ga, gb, c, hname):
                """sigmoid/tanh + cell update; returns new h tile"""
                sif = wk.tile([BC, 1024], F32, tag="sif", name=f"sif{hname}")
                nc.scalar.activation(sif, ga, AF.Sigmoid)
                tg = wk.tile([BC, 512], F32, tag="tg", name=f"tg{hname}")
                nc.scalar.activation(tg, gb[:, 512:1024], AF.Tanh)
                so = wk.tile([BC, 512], F32, tag="so", name=f"so{hname}")
                nc.scalar.activation(so, gb[:, 0:512], AF.Sigmoid)
                nc.vector.tensor_tensor(tg, sif[:, 0:512], tg, OP.mult)   # i*g
                nc.vector.tensor_tensor(c, sif[:, 512:1024], c, OP.mult)  # f*c
                nc.vector.tensor_tensor(c, c, tg, OP.add)
                tcell = wk.tile([BC, 512], F32, tag="tc", name=f"tc{hname}")
                nc.scalar.activation(tcell, c, AF.Tanh)
                h_new = wk.tile([BC, H], F32, tag=f"h{hname}", name=f"h{hname}_t")
                nc.vector.tensor_tensor(h_new, so, tcell, OP.mult)
                return h_new

            # ---- preamble: initial transposes + G0(0) ----
            h0T = transpose_h(h0, "0i")
            h1T = transpose_h(h1, "1i")
            ga0 = ps.tile([BC, 1024], F32, tag="ga", name="ga0_p")
            gb0 = ps.tile([BC, 1024], F32, tag="gb", name="gb0_p")
            gate_block(ga0, gb0, h0T, whh0, start=True, stop=False)

            for t in range(TS):
                tcols = slice(t * BC, (t + 1) * BC)

                # teacher forcing: z += (z==0)*mu_prev, in place on xt row 0
                if t > 0:
                    zrow = xt[0:1, tcols]
                    dlt = wk.tile([1, BC], F32, tag="dlt", name="dlt")
                    nc.vector.tensor_tensor(
                        dlt, mubuf[0:1, (t - 1) * BC:t * BC], iz[0:1, tcols], OP.mult
                    )
                    nc.vector.tensor_tensor(zrow, zrow, dlt, OP.add)
                xz_block(ga0, gb0, tcols)                      # G0(t) complete

                # G1(t) part A: Whh1 @ h1T(t-1) + bias1 (fills PE during acts0)
                ga1 = ps.tile([BC, 1024], F32, tag="ga", name="ga1")
                gb1 = ps.tile([BC, 1024], F32, tag="gb", name="gb1")
                gate_block(ga1, gb1, h1T, whh1, start=True, stop=False)
                rank1_block(ga1, gb1, ones1, b1)

                h0 = acts(ga0, gb0, c0, "0")
                h0T = transpose_h(h0, f"0_{t}")

                gate_block(ga1, gb1, h0T, wih1, start=False, stop=True)  # G1 done

                # G0(t+1): Whh0 @ h0T(t) (fills PE during acts1)
                if t + 1 < TS:
                    ga0 = ps.tile([BC, 1024], F32, tag="ga", name="ga0")
                    gb0 = ps.tile([BC, 1024], F32, tag="gb", name="gb0")
                    gate_block(ga0, gb0, h0T, whh0, start=True, stop=False)

                h1 = acts(ga1, gb1, c1, "1")
                h1T = transpose_h(h1, f"1_{t}")

                # head: [mu; ps] = 8 accumulating K=128,N=32 matmuls
                hd = ps.tile([2, BC], F32, tag="ga", name="hd",
                             padded_shape=[BC, 1024])
                for j in range(2 * KT):
                    hT = h0T if j < KT else h1T
                    rhs = hT[:, (j % KT) * BC:(j % KT + 1) * BC]
                    nc.tensor.matmul(
                        hd, hwc[:, j, :], rhs,
                        start=(j == 0), stop=(j == 2 * KT - 1),
                    )
                hdsb = wk.tile([2, BC], F32, tag="hdsb", name="hdsb")
                nc.vector.tensor_copy(hdsb, hd)
                nc.vector.tensor_scalar(mubuf[0:1, tcols], hdsb[0:1, :], hbmu, None, OP.add)
                nc.sync.dma_start(out=psraw_d[0:1, tcols], in_=hdsb[1:2, :])

            # ---- deferred loss math (Exp/Ln share one ACT table set) ----
            NCH = 8 if TS % 8 == 0 else 1
            SPC = TS // NCH          # steps per chunk
            W = SPC * BC
            sig_last = None
            for ci in range(NCH):
                cc = slice(ci * W, (ci + 1) * W)
                pb = fp.tile([1, W], F32, tag="fpb", name="pb")
                nc.sync.dma_start(out=pb, in_=psraw_d[0:1, cc])
                sig = fp.tile([1, W], F32, tag="fsig", name="sig")
                t1 = fp.tile([1, W], F32, tag="ft1", name="t1")
                # sigma = softplus(ps + b_ps) = ln(1 + exp(ps + b_ps))
                nc.scalar.activation(t1, pb, AF.Exp, bias=hbps[0:1, 0:1])
                nc.vector.tensor_scalar(t1, t1, 1.0, None, OP.add)
                nc.scalar.activation(sig, t1, AF.Ln)
                nc.vector.reciprocal(t1, sig)
                lb = fp.tile([1, W], F32, tag="flb", name="lb")
                nc.sync.dma_start(out=lb, in_=labs_d[0:1, cc])
                t2 = fp.tile([1, W], F32, tag="ft2", name="t2")
                nc.vector.tensor_tensor(t2, lb, mubuf[0:1, cc], OP.subtract)
                nc.vector.tensor_tensor(t2, t2, t1, OP.mult)
                nc.vector.tensor_tensor(t2, t2, t2, OP.mult)          # ((lab-mu)/sig)^2
                t3 = fp.tile([1, W], F32, tag="ft3", name="t3")
                nc.scalar.activation(t3, sig, AF.Ln)
                nc.vector.scalar_tensor_tensor(t2, t2, -0.5, t3, OP.mult, OP.subtract)
                nc.vector.tensor_scalar(t3, lb, 0.0, None, OP.not_equal)  # mask
                nc.vector.scalar_tensor_tensor(t2, t2, -0.5 * LOG2PI, t3, OP.add, OP.mult)
                csl = slice(ci * SPC, (ci + 1) * SPC)
                nc.vector.tensor_reduce(
                    ssum[0:1, csl], t2.rearrange("p (t b) -> p t b", b=BC),
                    mybir.AxisListType.X, OP.add,
                )
                nc.vector.tensor_reduce(
                    cnt[0:1, csl], t3.rearrange("p (t b) -> p t b", b=BC),
                    mybir.AxisListType.X, OP.add,
                )
                sig_last = sig

            # ---- outputs ----
            nc.sync.dma_start(out=ohc_d[0:32, :], in_=h0)
            nc.sync.dma_start(out=ohc_d[32:64, :], in_=h1)
            nc.sync.dma_start(out=ohc_d[64:96, :], in_=c0)
            nc.sync.dma_start(out=ohc_d[96:128, :], in_=c1)
            nc.sync.dma_start(out=oms_d[0:1, :], in_=mubuf[0:1, (TS - 1) * BC:TS * BC])
            nc.sync.dma_start(out=oms_d[1:2, :], in_=sig_last[0:1, (SPC - 1) * BC:SPC * BC])
            nc.sync.dma_start(out=olp_d[0:1, :], in_=ssum)
            nc.sync.dma_start(out=olp_d[1:2, :], in_=cnt)

    nc.compile()
    return nc


# ------------------------- host side -------------------------

def _pack_w(Wt_perm):
    """[512, 2048] (already col-permuted W.T) -> [128, 4, 2048] k-tile major"""
    return np.ascontiguousarray(
        Wt_perm.reshape(KT, 128, G).transpose(1, 0, 2), np.float32
    )


def host_prep(inputs, t_steps=T):
    f = lambda k: np.asarray(inputs[k], np.float32)
    train_batch, labels = f("train_batch"), f("labels_batch")
    hidden, cell, embed = f("hidden"), f("cell"), f("embed")
    W_ih0, W_hh0 = f("W_ih0"), f("W_hh0")
    W_ih1, W_hh1 = f("W_ih1"), f("W_hh1")
    b_ih0, b_hh0 = f("b_ih0"), f("b_hh0")
    b_ih1, b_hh1 = f("b_ih1"), f("b_hh1")
    W_mu, b_mu, W_ps, b_ps = f("W_mu"), f("b_mu"), f("W_ps"), f("b_ps")
    idx = np.asarray(inputs["idx"])
    TS = t_steps

    emb = embed[idx[0]]                                    # [B, 64]
    # head weights deinterleaved: [128, j, {mu,ps}]; j=0..3 h0 k-tiles, 4..7 h1
    hwc = np.zeros((128, 2 * KT, 2), np.float32)
    hwc[:, 0:KT, 0] = W_mu[0, 0::2].reshape(KT, 128).T
    hwc[:, KT:2 * KT, 0] = W_mu[0, 1::2].reshape(KT, 128).T
    hwc[:, 0:KT, 1] = W_ps[0, 0::2].reshape(KT, 128).T
    hwc[:, KT:2 * KT, 1] = W_ps[0, 1::2].reshape(KT, 128).T
    shared = {
        "whh0": _pack_w(np.ascontiguousarray(W_hh0.T[:, PERM])),
        "whh1": _pack_w(np.ascontiguousarray(W_hh1.T[:, PERM])),
        "wih1": _pack_w(np.ascontiguousarray(W_ih1.T[:, PERM])),
        "wxz": np.ascontiguousarray(
            np.concatenate([W_ih0.T[:, PERM], (b_ih0 + b_hh0)[None, PERM]], 0),
            np.float32),
        "b1": np.ascontiguousarray((b_ih1 + b_hh1)[None, PERM], np.float32),
        "hwc": hwc,
        "hbmu": b_mu.reshape(1, 1),
        "hbps": b_ps.reshape(1, 1),
    }

    in_maps = []
    for core in range(NCORES):
        sl = slice(core * BC, (core + 1) * BC)
        xc = train_batch[:TS, sl, :]                       # [TS, 32, 21]
        xt = np.empty((XR, TS, BC), np.float32)
        xt[0] = xc[:, :, 0]
        xt[1:21] = xc[:, :, 1:].transpose(2, 0, 1)
        xt[21:85] = np.broadcast_to(emb[sl].T[:, None, :], (64, TS, BC))
        xt[85] = 1.0
        m = dict(shared)
        m["ones1"] = np.ones((1, BC), np.float32)
        m["xt"] = np.ascontiguousarray(xt.reshape(XR, TS * BC))
        m["labs"] = np.ascontiguousarray(labels[:TS, sl].reshape(1, TS * BC))
        m["hc"] = np.ascontiguousarray(np.concatenate(
            [hidden[0, sl], hidden[1, sl], cell[0, sl], cell[1, sl]], 0))
        in_maps.append(m)
    return in_maps


def assemble(results, t_steps=T):
    mus = np.concatenate([r["out_musig"][0] for r in results]).astype(np.float32)
    sigmas = np.concatenate([r["out_musig"][1] for r in results]).astype(np.float32)
    h = np.stack([
        np.concatenate([r["out_hc"][0:32] for r in results], 0),
        np.concatenate([r["out_hc"][32:64] for r in results], 0),
    ]).astype(np.float32)
    c = np.stack([
        np.concatenate([r["out_hc"][64:96] for r in results], 0),
        np.concatenate([r["out_hc"][96:128] for r in results], 0),
    ]).astype(np.float32)
    S = np.sum([r["out_loss"][0] for r in results], axis=0, dtype=np.float32)
    CNT = np.sum([r["out_loss"][1] for r in results], axis=0, dtype=np.float32)
    n = np.maximum(CNT, 1.0).astype(np.float32)
    loss = np.float32(np.sum(-(S / n), dtype=np.float32))
    return mus, sigmas, h, c, np.array([loss], np.float32)


_CACHE = {}
LAST_EXEC_NS = None
_WARMED = False


def _run_with_retry(nc, in_maps, trace, attempts=4):
    import time

    from concourse.bass_utils import run_bass_kernel_spmd

    last = None
    for i in range(attempts):
        try:
            return run_bass_kernel_spmd(
                nc, in_maps, core_ids=list(range(NCORES)), trace=trace
            )
        except Exception as e:  # intermittent NRT_EXEC_UNIT_UNRECOVERABLE
            last = e
            sys.stderr.write(f"kernel run attempt {i} failed: {e}\n")
            time.sleep(3)
    raise last


def _warmup(inputs):
    """Run a tiny T=2 NEFF once; observed to stabilize the first big exec."""
    global _WARMED
    if _WARMED or os.environ.get("LSTM_WARMUP", "1") != "1":
        return
    try:
        nc2 = _CACHE.get("warm") or build_nc(2, mm_f32r=True)
        _CACHE["warm"] = nc2
        _run_with_retry(nc2, host_prep(inputs, 2), trace=False, attempts=2)
    except Exception as e:
        sys.stderr.write(f"warmup skipped: {e}\n")
    _WARMED = True


def kernel(**inputs):
    global LAST_EXEC_NS

    mm_f32r = os.environ.get("LSTM_MM_F32R", "1") == "1"
    trace = os.environ.get("LSTM_TRACE", "0") == "1"
    key = (T, mm_f32r)
    if key not in _CACHE:
        _CACHE[key] = build_nc(T, mm_f32r=mm_f32r)
    nc = _CACHE[key]
    in_maps = host_prep(inputs, T)
    _warmup(inputs)
    res = _run_with_retry(nc, in_maps, trace)
    LAST_EXEC_NS = res.exec_time_ns
    return assemble(res.results, T)
